# revision 1
# baseline (speedup 1.0000x reference)
"""Paged GQA attention Bass kernel for TRN2, SPMD over 8 cores (v3).

Sharding: tensor-parallel over KV heads. Core h owns KV head h and its 4
query heads. Per-core: B=4 seqs x S=2048 kv x (4 heads * 256 q) x d=128.

Data path (all indirection on device via the DMAGather ucode op):
  - K cache: one DMA-cast pass f32 HBM -> bf16 SBUF "slot table"
    [128 part, 88 ranks * 256B] (slot s -> partition s%128, rank s//128).
    Ranks 80..87 hold the new k tokens (decode-append invariant: new token
    j is position S-Q+(j%Q) of seq j//Q; checked host-side, with a host
    fallback rewrite if violated).
  - K^T per seq: one SBUF-source dma_gather (transpose=True) over 2048
    slot indices built on-device from block_tables -> [128 d, 2048 s] bf16.
    Seq 0 instead gathers token-major from the f32 cache + PE-transposes,
    so it does not wait for the staging pass.
  - V per seq: dma_gather (transpose=False) from the f32 HBM cache ->
    [128 tok, 14, 128 d], cast to bf16 on GPSIMD; last 2 tiles from v.
  - scores^T tile i = K^T[:,128i:128(i+1)].T @ Q^T -> exp (ACT, no
    max-subtraction; scores are O(5)) -> Et bf16 [128 s, 1024 (h,q)].
  - causal masks only where needed (host-built bf16 0/1, DVE mul).
  - out^T [128 d, 1024] += V_i.T @ Et_i in PSUM; denominator via DVE
    partial adds + ones-matmul; normalize after transposing back.
  - emission is software-pipelined: seq b+1's DMA prep is emitted before
    seq b's compute tiles so gathers overlap compute.
"""
import numpy as np
import ml_dtypes

import concourse.bass as bass
import concourse.bacc as bacc
import concourse.mybir as mybir
from concourse.tile import TileContext

F32 = mybir.dt.float32
BF16 = mybir.dt.bfloat16
I16 = mybir.dt.int16

B, Q, S = 4, 256, 2048
G, D = 4, 128
BLOCK = 16
NBULK_TOK = S - Q        # 1792 positions gathered from the cache per seq
NSLOT = 10240
NRANK = NSLOT // 128     # 80
NRANK_ALL = NRANK + (B * Q) // 128   # 88 with the new-k tail
QW = G * Q               # 1024
NT = S // 128            # 16 s-tiles per seq
SCALE = float(D) ** -0.5


def build_masks(seq_lens):
    mask_arrays, needs, cache = {}, {}, {}
    for b in range(B):
        sl = int(seq_lens[b])
        qpos = sl - Q + (np.arange(QW) % Q)
        for i in range(NT):
            kpos = 128 * i + np.arange(128)
            vis = (kpos[:, None] <= qpos[None, :]) & (kpos[:, None] < sl)
            if vis.all():
                needs[(b, i)] = None
                continue
            key = vis.tobytes()
            if key not in cache:
                name = f"mask{len(cache)}"
                cache[key] = name
                mask_arrays[name] = vis.astype(ml_dtypes.bfloat16)
            needs[(b, i)] = cache[key]
    return mask_arrays, needs


def build_nc(seq_lens=(2048,) * B, variant="full", repeat=1):
    nc = bacc.Bacc(None, target_bir_lowering=False, debug=False)

    q_ext = nc.declare_dram_parameter("q", [B * Q, G * D], F32, isOutput=False)
    k_ext = nc.declare_dram_parameter("k", [B * Q, D], F32, isOutput=False)
    v_ext = nc.declare_dram_parameter("v", [B * Q, D], F32, isOutput=False)
    kc = nc.declare_dram_parameter("kc", [NSLOT, D], F32, isOutput=False)
    vc = nc.declare_dram_parameter("vc", [NSLOT, D], F32, isOutput=False)
    bt16_ext = nc.declare_dram_parameter("bt16", [B, 128], I16, isOutput=False)
    idb_ext = nc.declare_dram_parameter("idb", [128, 128], BF16, isOutput=False)
    idf_ext = nc.declare_dram_parameter("idf", [128, 128], F32, isOutput=False)
    iota16_ext = nc.declare_dram_parameter("iota16b", [128, 112], I16, isOutput=False)
    tail0_ext = nc.declare_dram_parameter("tail0", [128, B * BLOCK], I16, isOutput=False)
    ones_ext = nc.declare_dram_parameter("onesb", [128, 1], BF16, isOutput=False)
    qidx_ext = nc.declare_dram_parameter("qidx", [128, B * BLOCK], I16, isOutput=False)

    mask_arrays, mask_needs = build_masks(seq_lens)
    mask_ext = {
        name: nc.declare_dram_parameter(name, [128, QW], BF16, isOutput=False)
        for name in mask_arrays
    }

    out_ext = nc.declare_dram_parameter("out", [B * Q, G * D], F32, isOutput=True)

    from contextlib import ExitStack

    with TileContext(nc) as tc, ExitStack() as stack:
        cpool = stack.enter_context(tc.tile_pool(name="consts", bufs=1))
        spool = stack.enter_context(tc.tile_pool(name="sbuf", bufs=3))
        idxpool = stack.enter_context(tc.tile_pool(name="idxp", bufs=3))
        et_pool = stack.enter_context(tc.tile_pool(name="et", bufs=8))
        ppool_sc = stack.enter_context(tc.tile_pool(name="psc", bufs=2, space="PSUM"))
        ppool_o = stack.enter_context(tc.tile_pool(name="po", bufs=1, space="PSUM"))
        ppool_t = stack.enter_context(tc.tile_pool(name="pt", bufs=2, space="PSUM"))

        # ---- constants ---- (small, index-path-critical tiles first so the
        # seq-0 index build is not queued behind the big constant loads)
        iota16b = cpool.tile([128, 112], I16, tag="iota16b")
        nc.sync.dma_start(out=iota16b[:], in_=iota16_ext[:, :])
        tail0 = cpool.tile([128, B * BLOCK], I16, tag="tail0")
        nc.sync.dma_start(out=tail0[:], in_=tail0_ext[:, :])
        onesb = cpool.tile([128, 1], BF16, tag="onesb")
        nc.sync.dma_start(out=onesb[:], in_=ones_ext[:, :])
        qidx = cpool.tile([128, B * BLOCK], I16, tag="qidx")
        nc.sync.dma_start(out=qidx[:], in_=qidx_ext[:, :])
        idb = cpool.tile([128, 128], BF16, tag="idb")
        nc.sync.dma_start(out=idb[:], in_=idb_ext[:, :])
        idf = cpool.tile([128, 128], F32, tag="idf")
        nc.sync.dma_start(out=idf[:], in_=idf_ext[:, :])
        masks = {}
        for name in mask_ext:
            m = cpool.tile([128, QW], BF16, tag=name)
            nc.sync.dma_start(out=m[:], in_=mask_ext[name][:, :])
            masks[name] = m

        if variant == "noop":
            z = spool.tile([128, 128], F32, tag="outsb")
            nc.vector.memset(z[:], 0.0)
            nc.sync.dma_start(out=out_ext[0:128, 0:128], in_=z[:])

        # PE clock warm-up: the HAM gate holds the PE at 1.2 GHz until it has
        # been busy for ~3.4us. The PE is idle during the startup gathers
        # anyway, so burn that window with dummy back-to-back matmuls (WAW on
        # one scratch tile keeps them serial) so the first QK tiles run warm.
        if variant != "noop":
            warm = ppool_t.tile([128, 128], F32, tag="tp", name="warm")
            for _w in range(28):
                nc.tensor.matmul(warm[:], lhsT=idb[:], rhs=idb[:],
                                 start=True, stop=True)

        kcb = cpool.tile([128, NRANK_ALL * D], BF16, tag="kcb")
        kcb_v = kcb[:].rearrange("p (r d) -> p r d", r=NRANK_ALL, d=D)
        qcb = cpool.tile([128, (B * Q // 128) * G * D], BF16, tag="qcb")
        qcb_v = qcb[:].rearrange("p (r hd) -> p r hd", r=B * Q // 128, hd=G * D)

        def emit_idx(b):
            idx16 = idxpool.tile([128, NT * BLOCK // 2], I16, tag="idx16")
            bt_b = idxpool.tile([128, 112], I16, tag="bt_b")
            nc.sync.dma_start(
                out=bt_b[:], in_=bt16_ext[b : b + 1, 0:112].to_broadcast((128, 112))
            )
            nc.vector.tensor_scalar(
                out=idx16[:, 0:112], in0=bt_b[:], scalar1=4, scalar2=None,
                op0=mybir.AluOpType.arith_shift_left,
            )
            nc.vector.tensor_tensor(
                out=idx16[:, 0:112], in0=idx16[:, 0:112], in1=iota16b[:],
                op=mybir.AluOpType.add,
            )
            nc.vector.tensor_copy(
                idx16[:, 112:128], tail0[:, b * BLOCK : (b + 1) * BLOCK]
            )
            return idx16

        def emit_prep_dma(b, fast_start):
            """Index build + gathers + raw input loads for seq b."""
            st = {}
            idx16 = emit_idx(b)
            st["kt"] = spool.tile([128, S], BF16, tag="kt", name="kt")
            st["vg"] = spool.tile([128, 14 * D], F32, tag="vg", name="vg")
            nc.gpsimd.dma_gather(
                out_ap=st["vg"][:].rearrange("p (c d) -> p c d", d=D),
                in_ap=vc[:, :], idxs_ap=idx16[:, 0:112],
                num_idxs=NBULK_TOK, num_idxs_reg=NBULK_TOK, elem_size=D,
                transpose=False, single_packet=False,
            )

            if fast_start:
                st["qsb"] = cpool.tile([128, 2 * G * D], F32, tag="qsb", name="qsb")
                nc.sync.dma_start(
                    out=st["qsb"][:].rearrange("p (t hd) -> p t hd", t=2, hd=G * D),
                    in_=q_ext[b * Q : (b + 1) * Q, :].rearrange(
                        "(t p) hd -> p t hd", t=2, p=128
                    ),
                )
                st["kg0"] = cpool.tile([128, 14 * D], F32, tag="kg0", name="kg0")
                nc.gpsimd.dma_gather(
                    out_ap=st["kg0"][:].rearrange("p (c d) -> p c d", d=D),
                    in_ap=kc[:, :], idxs_ap=idx16[:, 0:112],
                    num_idxs=NBULK_TOK, num_idxs_reg=NBULK_TOK, elem_size=D,
                    transpose=False, single_packet=False,
                )
                st["ktl"] = cpool.tile([128, 2 * D], F32, tag="ktl", name="ktl")
                nc.sync.dma_start(
                    out=st["ktl"][:].rearrange("p (t d) -> p t d", t=2, d=D),
                    in_=k_ext[b * Q : (b + 1) * Q, :].rearrange(
                        "(t p) d -> p t d", t=2, p=128
                    ),
                )
            else:
                nc.gpsimd.dma_gather(
                    out_ap=st["kt"][:].rearrange("p (a n) -> p a n", a=1),
                    in_ap=kcb[:], idxs_ap=idx16[:],
                    num_idxs=S, num_idxs_reg=S, elem_size=D,
                    transpose=True, single_packet=False,
                    sbuf_tokens_per_rank=128, sbuf_free_dim_per_rank=D * 2,
                )
            st["vtl"] = spool.tile([128, 2 * D], F32, tag="vtl", name="vtl")
            nc.sync.dma_start(
                out=st["vtl"][:].rearrange("p (t d) -> p t d", t=2, d=D),
                in_=v_ext[b * Q : (b + 1) * Q, :].rearrange(
                    "(t p) d -> p t d", t=2, p=128
                ),
            )
            return st

        def emit_prep_compute(b, st, fast_start):
            """Casts + transposes producing kt/vt/qt for seq b."""
            eng = nc.vector if fast_start else nc.gpsimd
            vt = spool.tile([128, S], BF16, tag="vt")
            eng.tensor_copy(vt[:, 0:NBULK_TOK], st["vg"][:])
            eng.tensor_copy(vt[:, NBULK_TOK:S], st["vtl"][:])
            st["vt"] = vt
            # Q^T [128 d, (h, qt)] via SBUF-source transposed gather over the
            # staged bf16 q; gathered layout is [d, h, qt] = (h, qt) packed.
            qt_t = spool.tile([128, QW], BF16, tag="qt")
            if fast_start:
                qsbb = cpool.tile([128, 2 * G * D], BF16, tag="qsbb")
                nc.vector.tensor_copy(qsbb[:], st["qsb"][:])
                for h in range(G):
                    for t in range(2):
                        ps = ppool_t.tile([128, 128], BF16, tag="tp")
                        nc.tensor.transpose(
                            ps[:],
                            qsbb[:, t * G * D + h * D : t * G * D + (h + 1) * D],
                            idb[:],
                        )
                        nc.vector.tensor_copy(
                            qt_t[:, h * Q + t * 128 : h * Q + (t + 1) * 128], ps[:]
                        )
            else:
                nc.gpsimd.dma_gather(
                    out_ap=qt_t[:].rearrange("p (h n) -> p h n", h=G, n=Q),
                    in_ap=qcb[:], idxs_ap=qidx[:, b * BLOCK : (b + 1) * BLOCK],
                    num_idxs=Q, num_idxs_reg=Q, elem_size=G * D,
                    transpose=True, single_packet=True,
                    sbuf_tokens_per_rank=128, sbuf_free_dim_per_rank=G * D * 2,
                )
            st["qt"] = qt_t
            if fast_start:
                kg0b = cpool.tile([128, NBULK_TOK], BF16, tag="kg0b")
                nc.vector.tensor_copy(kg0b[:], st["kg0"][:])
                ktlb = cpool.tile([128, 2 * D], BF16, tag="ktlb")
                nc.vector.tensor_copy(ktlb[:], st["ktl"][:])
                for c in range(NT):
                    ps = ppool_t.tile([128, 128], BF16, tag="tp")
                    src = (
                        kg0b[:, c * D : (c + 1) * D]
                        if c < 14
                        else ktlb[:, (c - 14) * D : (c - 13) * D]
                    )
                    nc.tensor.transpose(ps[:], src, idb[:])
                    nc.vector.tensor_copy(
                        st["kt"][:, c * 128 : (c + 1) * 128], ps[:]
                    )

        def emit_compute(b, st, mid_hook=None):
            partial = spool.tile([128, QW], BF16, tag="partial")
            psum_o = ppool_o.tile([128, QW], F32, tag="po")
            kt, vt, qt_t = st["kt"], st["vt"], st["qt"]
            # PV is software-pipelined one tile behind QK/exp so the
            # in-order PE engine can issue QK(i+1) during exp(i)'s latency
            # instead of stalling on PV(i)'s wait for the ACT result.
            def emit_pv(i, et):
                v_tile = vt[:, i * 128 : (i + 1) * 128]
                for half in range(2):
                    nc.tensor.matmul(
                        psum_o[:, half * 512 : (half + 1) * 512],
                        lhsT=v_tile,
                        rhs=et[:, half * 512 : (half + 1) * 512],
                        start=(i == 0), stop=(i == NT - 1),
                    )

            prev_et = None
            for i in range(NT):
                if i == 6 and mid_hook is not None:
                    mid_hook()
                kt_cols = kt[:, i * 128 : (i + 1) * 128]
                psc = ppool_sc.tile([128, QW], F32, tag="psc")
                for half in range(2):
                    nc.tensor.matmul(
                        psc[:, half * 512 : (half + 1) * 512],
                        lhsT=kt_cols,
                        rhs=qt_t[:, half * 512 : (half + 1) * 512],
                        start=True, stop=True,
                    )
                et = et_pool.tile([128, QW], BF16, tag="et")
                nc.scalar.activation(
                    et[:], psc[:], mybir.ActivationFunctionType.Exp, scale=SCALE
                )
                mname = mask_needs[(b, i)]
                if mname is not None:
                    nc.vector.tensor_mul(et[:], et[:], masks[mname][:])
                if i == 0:
                    nc.vector.tensor_copy(partial[:], et[:])
                else:
                    nc.vector.tensor_add(partial[:], partial[:], et[:])
                if prev_et is not None:
                    emit_pv(i - 1, prev_et)
                prev_et = et
            emit_pv(NT - 1, prev_et)
            osb = spool.tile([128, QW], BF16, tag="osb")
            nc.vector.tensor_copy(osb[:], psum_o[:])
            st["partial"], st["osb"] = partial, osb

        def emit_finalize(b, st):
            partial, osb = st["partial"], st["osb"]
            den_ps = ppool_sc.tile([1, QW], F32, tag="psc")
            for half in range(2):
                nc.tensor.matmul(
                    den_ps[:, half * 512 : (half + 1) * 512],
                    lhsT=onesb[:],
                    rhs=partial[:, half * 512 : (half + 1) * 512],
                    start=True, stop=True,
                )
            den_sb = spool.tile([1, QW], F32, tag="densb")
            nc.vector.tensor_copy(den_sb[:], den_ps[:])
            rp_ps = ppool_t.tile([128, 8], F32, tag="tp")
            for j in range(8):
                nc.tensor.transpose(
                    rp_ps[:, j : j + 1], den_sb[0:1, j * 128 : (j + 1) * 128],
                    idf[0:1, 0:1],
                )
            rp_sb = spool.tile([128, 8], F32, tag="rpsb")
            nc.vector.tensor_copy(rp_sb[:], rp_ps[:])
            recip = spool.tile([128, 8], F32, tag="recip")
            nc.vector.reciprocal(recip[:], rp_sb[:])

            for j in range(8):
                ps = ppool_t.tile([128, 128], BF16, tag="tp")
                nc.tensor.transpose(ps[:], osb[:, j * 128 : (j + 1) * 128], idb[:])
                o_sb = spool.tile([128, 128], F32, tag="outsb")
                nc.vector.tensor_scalar(
                    out=o_sb[:], in0=ps[:], scalar1=recip[:, j : j + 1],
                    scalar2=None, op0=mybir.AluOpType.mult,
                )
                h, tt = j // 2, j % 2
                nc.sync.dma_start(
                    out=out_ext[
                        b * Q + tt * 128 : b * Q + (tt + 1) * 128,
                        h * D : (h + 1) * D,
                    ],
                    in_=o_sb[:],
                )

        for _rep in range(repeat if variant != "noop" else 0):
            first = _rep == 0
            # seq 0 DMA prep first (its K path reads the raw f32 cache), then
            # the staging pass, then seq 1's gathers queue behind staging.
            st = {0: emit_prep_dma(0, fast_start=first)}
            emit_prep_compute(0, st[0], fast_start=first)
            if first:
                nc.gpsimd.dma_start(
                    out=qcb_v[:, :, :],
                    in_=q_ext.rearrange("(r p) hd -> p r hd", p=128),
                )
                kc_v = kc.rearrange("(r p) d -> p r d", p=128)
                for r0 in range(0, NRANK, 10):
                    nc.gpsimd.dma_start(
                        out=kcb_v[:, r0 : r0 + 10, :],
                        in_=kc_v[:, r0 : r0 + 10, :],
                    )
                nc.gpsimd.dma_start(
                    out=kcb_v[:, NRANK:NRANK_ALL, :],
                    in_=k_ext.rearrange("(r p) d -> p r d", p=128),
                )
            for b in range(B):
                if b + 1 < B:
                    st[b + 1] = emit_prep_dma(b + 1, fast_start=False)
                if b - 1 >= 0:
                    fb = b - 1
                    hook = lambda fb=fb: (emit_finalize(fb, st[fb]), st.pop(fb))
                else:
                    hook = None
                emit_compute(b, st[b], mid_hook=hook)
                if b + 1 < B:
                    emit_prep_compute(b + 1, st[b + 1], fast_start=False)
            emit_finalize(B - 1, st[B - 1])

    nc.finalize()
    return nc, mask_arrays


def make_consts():
    idb = np.eye(128).astype(ml_dtypes.bfloat16)
    idf = np.eye(128, dtype=np.float32)
    p = np.arange(128) % 16
    iota16b = np.tile(p[:, None], (1, 112)).astype(np.int16)
    bc = np.arange(B * BLOCK)
    tail0 = (NSLOT + (bc // BLOCK)[None, :] * Q + (bc % BLOCK)[None, :] * 16
             + p[:, None]).astype(np.int16)
    onesb = np.ones((128, 1), ml_dtypes.bfloat16)
    qidx = ((bc // BLOCK)[None, :] * Q // BLOCK * BLOCK + (bc % BLOCK)[None, :] * 16
            + p[:, None]).astype(np.int16)
    qidx = (((bc // BLOCK) * Q)[None, :] * 0 + 0).astype(np.int16)  # placeholder
    qidx = np.zeros((128, B * BLOCK), np.int16)
    for b in range(B):
        for c in range(BLOCK):
            for r in range(128):
                qidx[r, b * BLOCK + c] = b * Q + c * 16 + (r % 16)
    return dict(idb=idb, idf=idf, iota16b=iota16b, tail0=tail0, onesb=onesb,
                qidx=qidx)


def check_invariant(slot_mapping, block_tables):
    pos = np.arange(S - Q, S)
    want = (block_tables[:, pos // BLOCK] * BLOCK + pos % BLOCK).reshape(-1)
    return np.array_equal(slot_mapping.reshape(-1), want)


def shard_inputs(q, k, v, kv_cache, slot_mapping, block_tables, seq_lens,
                 query_start_loc, mask_arrays):
    consts = make_consts()
    bt16 = np.ascontiguousarray(block_tables).astype(np.int16)
    kv_cache = np.asarray(kv_cache)
    k_use, v_use = k, v
    kc_all = kv_cache[0]
    vc_all = kv_cache[1]
    if not check_invariant(slot_mapping, block_tables):
        # Generic fallback: apply the scatter host-side, then read the
        # boundary region back so the kernel's append-tail sees the true
        # post-scatter cache contents.
        kc_all = kc_all.copy().reshape(NSLOT, 8, D)
        vc_all = vc_all.copy().reshape(NSLOT, 8, D)
        sm = np.asarray(slot_mapping).reshape(-1)
        kc_all[sm] = np.asarray(k).reshape(-1, 8, D)
        vc_all[sm] = np.asarray(v).reshape(-1, 8, D)
        pos = np.arange(S - Q, S)
        slots_b = (block_tables[:, pos // BLOCK] * BLOCK + pos % BLOCK).reshape(-1)
        k_use = kc_all[slots_b].reshape(B * Q, 8 * D)
        v_use = vc_all[slots_b].reshape(B * Q, 8 * D)
        kc_all = kc_all.reshape(640, BLOCK, 8, D)
        vc_all = vc_all.reshape(640, BLOCK, 8, D)
    in_maps = []
    for h in range(8):
        m = {
            "q": np.ascontiguousarray(q[:, h * G * D : (h + 1) * G * D]),
            "k": np.ascontiguousarray(k_use[:, h * D : (h + 1) * D]),
            "v": np.ascontiguousarray(v_use[:, h * D : (h + 1) * D]),
            "kc": np.ascontiguousarray(kc_all[:, :, h, :]).reshape(NSLOT, D),
            "vc": np.ascontiguousarray(vc_all[:, :, h, :]).reshape(NSLOT, D),
            "bt16": bt16,
            **consts,
            **mask_arrays,
        }
        in_maps.append(m)
    return in_maps


def assemble_output(results):
    return np.concatenate([results[h]["out"] for h in range(8)], axis=1)


# ---------------------------------------------------------------------------
# Harness entry point: kernel(**inputs) with FULL (unsharded) inputs.
# ---------------------------------------------------------------------------
from concourse.bass_utils import run_bass_kernel_spmd

_CACHE = {}


def _get_nc(seq_lens):
    key = tuple(int(x) for x in seq_lens)
    if key not in _CACHE:
        _CACHE[key] = build_nc(key)
    return _CACHE[key]


def kernel(q, k, v, kv_cache, slot_mapping, block_tables, seq_lens,
           query_start_loc, **extra):
    q = np.asarray(q); k = np.asarray(k); v = np.asarray(v)
    kv_cache = np.asarray(kv_cache)
    slot_mapping = np.asarray(slot_mapping)
    block_tables = np.asarray(block_tables)
    seq_lens = np.asarray(seq_lens)
    nc, mask_arrays = _get_nc(seq_lens)
    in_maps = shard_inputs(q, k, v, kv_cache, slot_mapping, block_tables,
                           seq_lens, query_start_loc, mask_arrays)
    res = run_bass_kernel_spmd(nc, in_maps, core_ids=list(range(8)))
    return assemble_output(res.results)

